# revision 1
# baseline (speedup 1.0000x reference)
"""Trainium2 Bass kernel for nn_Attention_31267361915369.

Computation (per batch example, T=4096, D=1024):
    h   = tanh(x @ W1.T + b1)          # [T, D]
    s   = h @ w2.T + b2                # [T]
    e   = exp(s)                       # no max-subtraction: |s| <= sum|w2| ~ 26,
                                       # and num/den is exactly shift-invariant
    num = cumsum(e * x, axis=0)        # [T, D]
    den = cumsum(e)                    # [T]
    ctx = num / den
    out = tanh([ctx, x] @ Wc.T + bc)   # [T, D]

Distribution: data-parallel over batch B=8 across the 8 NeuronCores (one
example per core), weights replicated. No collectives.

Per-core dataflow (32 token-tiles of 128):
  - x tile DMA'd in natural [t, d] layout (bf16), PE-transposed to [d, t]
    blocks for the matmuls (TensorE contracts over the partition dim).
  - all matmuls run in bf16 (1 cycle/row on TensorE vs 4 for fp32) with
    fp32 PSUM accumulation; measured end-to-end rel err ~2e-3.
  - the causal prefix sums run on TensorE as a [128,128] upper-triangular
    ones matmul per tile; the running carry is extracted same-partition
    (row 127 of PSUM) into a carry tile and broadcast back to all 128
    partitions with a selector matmul (sel[127,:] = 1).
  - stage-skewed emission: tile i's score pipeline (ACT/DVE) overlaps
    tile i+1's transposes/matmul1 on TensorE.
"""

import sys

if "/opt/trn_rl_repo" not in sys.path:
    sys.path.insert(0, "/opt/trn_rl_repo")

from contextlib import ExitStack

import ml_dtypes
import numpy as np

import concourse.bass as bass
import concourse.tile as tile
from concourse import bacc, mybir
from concourse.bass_utils import run_bass_kernel_spmd

P = 128
D = 1024
T_FULL = 4096
N_CORES = 8

BF = mybir.dt.bfloat16
F32 = mybir.dt.float32
AFT = mybir.ActivationFunctionType
ALU = mybir.AluOpType

_BUILD_CACHE: dict = {}


def build(T: int = T_FULL, use_b1: bool = False, use_bc: bool = False,
          repeat: int = 1, xt_host: bool = True, carry_v2: bool = True):
    """Build + compile the per-core Bass program for a [T, D] shard.

    repeat>1 wraps the whole pipeline in a hardware For_i loop that
    recomputes the same output `repeat` times — used only for timing
    (slope over repeat isolates device compute from dispatch noise).
    """
    key = (T, use_b1, use_bc, repeat, xt_host, carry_v2)
    if key in _BUILD_CACHE:
        return _BUILD_CACHE[key]

    assert T % P == 0
    NT = T // P
    NK1 = D // P          # 8 k-tiles for matmul1
    NK3 = 2 * D // P      # 16 k-tiles for matmul3

    nc = bacc.Bacc("TRN2", target_bir_lowering=False, debug=False)

    x_d = nc.declare_dram_parameter("x", [T, D], BF, isOutput=False)
    # host-pretransposed x: xt[i, p, k, t] = x[i*128+t, k*128+p]
    xt_d = (nc.declare_dram_parameter("xt", [T * (D // P), P], BF, isOutput=False)
            if xt_host else None)
    w1t_d = nc.declare_dram_parameter("w1t", [D, D], BF, isOutput=False)
    wct_d = nc.declare_dram_parameter("wct", [2 * D, D], BF, isOutput=False)
    w2r_d = nc.declare_dram_parameter("w2r", [P, D], F32, isOutput=False)
    tri_d = nc.declare_dram_parameter("tri", [P, P], BF, isOutput=False)
    idn_d = nc.declare_dram_parameter("idn", [P, P], BF, isOutput=False)
    sel_d = nc.declare_dram_parameter("sel", [P, P], BF, isOutput=False)
    b1_d = nc.declare_dram_parameter("b1r", [1, D], BF, isOutput=False) if use_b1 else None
    bc_d = nc.declare_dram_parameter("bcr", [1, D], BF, isOutput=False) if use_bc else None
    out_d = nc.declare_dram_parameter("out", [T, D], F32, isOutput=True)

    x_t = x_d.ap().rearrange("(n p) d -> n p d", p=P)
    xt_t = (xt_d.ap().rearrange("(n p k) q -> n p k q", p=P, k=D // P)
            if xt_host else None)
    out_t = out_d.ap().rearrange("(n p) d -> n p d", p=P)
    w1_t = w1t_d.ap().rearrange("(k p) e -> k p e", p=P)
    wc_t = wct_d.ap().rearrange("(k p) e -> k p e", p=P)

    with tile.TileContext(nc) as tc, ExitStack() as ctx:
        consts = ctx.enter_context(tc.tile_pool(name="consts", bufs=1))
        xin = ctx.enter_context(tc.tile_pool(name="xin", bufs=3))
        xtp = ctx.enter_context(tc.tile_pool(name="xtp", bufs=3))
        hpool = ctx.enter_context(tc.tile_pool(name="hpool", bufs=2))
        scr = ctx.enter_context(tc.tile_pool(name="scr", bufs=2))
        expool = ctx.enter_context(tc.tile_pool(name="expool", bufs=2))
        ctxp = ctx.enter_context(tc.tile_pool(name="ctxp", bufs=2))
        ctxtp = ctx.enter_context(tc.tile_pool(name="ctxtp", bufs=2))
        outp = ctx.enter_context(tc.tile_pool(name="outp", bufs=2))
        colp = ctx.enter_context(tc.tile_pool(name="colp", bufs=4))
        carryp = ctx.enter_context(tc.tile_pool(name="carryp", bufs=2))
        crowp = ctx.enter_context(tc.tile_pool(name="crowp", bufs=2))
        ph = ctx.enter_context(tc.tile_pool(name="ph", bufs=1, space="PSUM"))
        pc = ctx.enter_context(tc.tile_pool(name="pc", bufs=1, space="PSUM"))
        if xt_host:
            pt2 = ctx.enter_context(tc.tile_pool(name="pt2", bufs=2, space="PSUM"))
        else:
            pt1 = ctx.enter_context(tc.tile_pool(name="pt1", bufs=1, space="PSUM"))
            pt2 = ctx.enter_context(tc.tile_pool(name="pt2", bufs=1, space="PSUM"))
        po = ctx.enter_context(tc.tile_pool(name="po", bufs=1, space="PSUM"))

        # constants / weights (small ones first: needed earliest)
        tri_sb = consts.tile([P, P], BF, tag="tri")
        nc.sync.dma_start(out=tri_sb[:], in_=tri_d.ap())
        idn_sb = consts.tile([P, P], BF, tag="idn")
        nc.sync.dma_start(out=idn_sb[:], in_=idn_d.ap())
        sel_sb = consts.tile([P, P], BF, tag="sel")
        nc.sync.dma_start(out=sel_sb[:], in_=sel_d.ap())
        # f32: wide bf16 TensorTensor/TensorReduce DVE ops hang on this hw
        w2r_sb = consts.tile([P, D], F32, tag="w2r")
        nc.sync.dma_start(out=w2r_sb[:], in_=w2r_d.ap())
        if use_b1:
            b1_sb = consts.tile([1, D], BF, tag="b1")
            nc.sync.dma_start(out=b1_sb[:], in_=b1_d.ap())
        if use_bc:
            bc_sb = consts.tile([1, D], BF, tag="bc")
            nc.sync.dma_start(out=bc_sb[:], in_=bc_d.ap())
        w1_sb = []
        for k in range(NK1):
            t = consts.tile([P, D], BF, tag=f"w1_{k}")
            nc.sync.dma_start(out=t[:], in_=w1_t[k])
            w1_sb.append(t)
        wc_sb = []
        for k in range(NK3):
            t = consts.tile([P, D], BF, tag=f"wc_{k}")
            nc.sync.dma_start(out=t[:], in_=wc_t[k])
            wc_sb.append(t)

        carry_tiles = {}

        def stage_a(i):
            """load + transpose + scores for tile i -> (xT, ex)"""
            x_sb = xin.tile([P, D], BF, tag="x")
            nc.sync.dma_start(out=x_sb[:], in_=x_t[i])

            if xt_host:
                xT = xtp.tile([P, NK1, P], BF, tag="xt")
                nc.sync.dma_start(out=xT[:], in_=xt_t[i])
            else:
                ptile = pt1.tile([P, D], BF, tag="pt1")
                for k in range(NK1):
                    nc.tensor.transpose(
                        ptile[:, k * P:(k + 1) * P], x_sb[:, k * P:(k + 1) * P],
                        idn_sb[:],
                    )
                xT = xtp.tile([P, NK1, P], BF, tag="xt")
                nc.vector.tensor_copy(
                    xT[:], ptile[:].rearrange("p (k q) -> p k q", k=NK1)
                )

            ph_t = ph.tile([P, D], F32, tag="ph")
            for k in range(NK1):
                last = k == NK1 - 1 and not use_b1
                for c in range(2):
                    nc.tensor.matmul(
                        ph_t[:, c * 512:(c + 1) * 512],
                        xT[:, k, :],
                        w1_sb[k][:, c * 512:(c + 1) * 512],
                        start=(k == 0),
                        stop=last,
                    )
            if use_b1:
                for c in range(2):
                    nc.tensor.matmul(
                        ph_t[:, c * 512:(c + 1) * 512],
                        tri_sb[0:1, :],
                        b1_sb[0:1, c * 512:(c + 1) * 512],
                        start=False,
                        stop=True,
                    )
            h_sb = hpool.tile([P, D], F32, tag="h")
            nc.scalar.activation(h_sb[:], ph_t[:], AFT.Tanh)

            s_col = colp.tile([P, 1], F32, tag="s")
            prod = scr.tile([P, D], F32, tag="scr")
            nc.vector.tensor_mul(prod[:], h_sb[:], w2r_sb[:])
            nc.vector.reduce_sum(s_col[:], prod[:], axis=mybir.AxisListType.X)
            e_col = colp.tile([P, 1], F32, tag="e")
            nc.scalar.activation(e_col[:], s_col[:], AFT.Exp)
            ex_sb = expool.tile([P, D + 1], BF, tag="ex")
            nc.scalar.copy(ex_sb[:, D:D + 1], e_col[:])
            nc.vector.tensor_scalar_mul(ex_sb[:, 0:D], x_sb[:], e_col[:])
            return xT, ex_sb

        def stage_b(i, xT, ex_sb):
            """cumsum + ctx + output matmul for tile i"""
            if carry_v2 and i > 0:
                # inject the running carry into ex row 0: U[0, t] = 1 for all
                # t, so the triangular matmul propagates it to every output
                # row — replaces the 3 sel broadcast matmuls.
                nc.vector.tensor_add(
                    ex_sb[0:1, :], ex_sb[0:1, :], carry_tiles[i - 1][0:1, :]
                )
            close = carry_v2 or i == 0
            pc_t = pc.tile([P, D], F32, tag="pc")
            pd_t = po.tile([P, 1], F32, tag="po")
            for c in range(2):
                nc.tensor.matmul(
                    pc_t[:, c * 512:(c + 1) * 512],
                    tri_sb[:],
                    ex_sb[:, c * 512:(c + 1) * 512],
                    start=True,
                    stop=close,
                )
            nc.tensor.matmul(
                pd_t[:], tri_sb[:], ex_sb[:, D:D + 1], start=True, stop=close
            )
            if not carry_v2 and i > 0:
                cprev = carry_tiles[i - 1]
                for c in range(2):
                    nc.tensor.matmul(
                        pc_t[:, c * 512:(c + 1) * 512],
                        sel_sb[:],
                        cprev[:, c * 512:(c + 1) * 512],
                        start=False,
                        stop=True,
                    )
                nc.tensor.matmul(
                    pd_t[:], sel_sb[:], cprev[:, D:D + 1], start=False, stop=True
                )

            # extract running totals (row 127 of PSUM) for the next tile's
            # carry. engines can't move data across partitions (and must start
            # at a 32-aligned partition), so copy the [96:128] window.
            if i < NT - 1:
                if carry_v2:
                    cstage = carryp.tile([P, D + 1], BF, tag="carry")
                    nc.scalar.copy(cstage[96:128, 0:D], pc_t[96:128, :])
                    nc.scalar.copy(cstage[96:128, D:D + 1], pd_t[96:128, :])
                    crow = crowp.tile([1, D + 1], BF, tag="crow")
                    nc.sync.dma_start(out=crow[0:1, :], in_=cstage[127:128, :])
                    carry_tiles[i] = crow
                else:
                    cnew = carryp.tile([P, D + 1], BF, tag="carry")
                    nc.vector.memset(cnew[0:96, :], 0.0)
                    nc.scalar.copy(cnew[96:128, 0:D], pc_t[96:128, :])
                    nc.scalar.copy(cnew[96:128, D:D + 1], pd_t[96:128, :])
                    carry_tiles[i] = cnew

            rden = colp.tile([P, 1], F32, tag="rden")
            nc.vector.reciprocal(rden[:], pd_t[:])
            ctx_sb = ctxp.tile([P, D], BF, tag="ctx")
            nc.vector.tensor_scalar_mul(ctx_sb[:], pc_t[:], rden[:])

            ptile = pt2.tile([P, D], BF, tag="pt2")
            for k in range(NK1):
                nc.tensor.transpose(
                    ptile[:, k * P:(k + 1) * P], ctx_sb[:, k * P:(k + 1) * P], idn_sb[:]
                )
            ctxT = ctxtp.tile([P, D], BF, tag="ctxt")
            nc.scalar.copy(ctxT[:], ptile[:])

            po_t = po.tile([P, D], F32, tag="po")
            for k in range(NK3):
                if k < NK1:
                    lhsT = ctxT[:, k * P:(k + 1) * P]
                else:
                    lhsT = xT[:, k % NK1, :]
                last = k == NK3 - 1 and not use_bc
                for c in range(2):
                    nc.tensor.matmul(
                        po_t[:, c * 512:(c + 1) * 512],
                        lhsT,
                        wc_sb[k][:, c * 512:(c + 1) * 512],
                        start=(k == 0),
                        stop=last,
                    )
            if use_bc:
                for c in range(2):
                    nc.tensor.matmul(
                        po_t[:, c * 512:(c + 1) * 512],
                        tri_sb[0:1, :],
                        bc_sb[0:1, c * 512:(c + 1) * 512],
                        start=False,
                        stop=True,
                    )
            o_sb = outp.tile([P, D], F32, tag="out")
            nc.scalar.activation(o_sb[:], po_t[:], AFT.Tanh)
            nc.sync.dma_start(out=out_t[i], in_=o_sb[:])

        def whole_pipeline():
            carry_tiles.clear()
            pend = None
            for i in range(NT):
                cur = stage_a(i)
                if pend is not None:
                    stage_b(i - 1, *pend)
                pend = cur
            stage_b(NT - 1, *pend)

        if repeat == 1:
            whole_pipeline()
        else:
            with tc.For_i(0, repeat, 1):
                whole_pipeline()

    nc.compile()
    _BUILD_CACHE[key] = nc
    return nc


def _bf16(a):
    return np.ascontiguousarray(np.asarray(a, dtype=np.float32)).astype(
        ml_dtypes.bfloat16
    )


def make_in_maps(x, W1, b1, w2, b2, Wc, bc, T=T_FULL):
    """Host-side prep: shard x over batch, pre-transpose/replicate weights."""
    x = np.asarray(x, dtype=np.float32)
    W1 = np.asarray(W1, dtype=np.float32)
    Wc = np.asarray(Wc, dtype=np.float32)
    w2 = np.asarray(w2, dtype=np.float32).reshape(1, -1)
    b1 = np.asarray(b1, dtype=np.float32)
    bc = np.asarray(bc, dtype=np.float32)
    use_b1 = bool(np.any(b1 != 0.0))
    use_bc = bool(np.any(bc != 0.0))
    # b2 shifts every score equally; exp(b2) cancels in num/den.

    w1t = _bf16(W1.T)
    wct = _bf16(Wc.T)
    w2r = np.ascontiguousarray(np.broadcast_to(w2, (P, D)).astype(np.float32))
    tri = _bf16(np.triu(np.ones((P, P), np.float32)))
    idn = _bf16(np.eye(P, dtype=np.float32))
    sel = np.zeros((P, P), np.float32)
    sel[P - 1, :] = 1.0
    sel = _bf16(sel)

    NT = T // P
    NK = D // P
    in_maps = []
    for i in range(N_CORES):
        xb = _bf16(x[i, :T, :])
        # xt[i, p, k, t] = x[i*128+t, k*128+p], 2KB-contiguous per partition
        xt = np.ascontiguousarray(
            xb.reshape(NT, P, NK, P).transpose(0, 3, 2, 1)
        ).reshape(T * NK, P)
        m = {
            "x": xb,
            "xt": xt,
            "w1t": w1t,
            "wct": wct,
            "w2r": w2r,
            "tri": tri,
            "idn": idn,
            "sel": sel,
        }
        if use_b1:
            m["b1r"] = _bf16(b1.reshape(1, D))
        if use_bc:
            m["bcr"] = _bf16(bc.reshape(1, D))
        in_maps.append(m)
    return in_maps, use_b1, use_bc


def kernel(x, W1, b1, w2, b2, Wc, bc):
    in_maps, use_b1, use_bc = make_in_maps(x, W1, b1, w2, b2, Wc, bc)
    nc = build(T_FULL, use_b1, use_bc)
    res = run_bass_kernel_spmd(nc, in_maps, core_ids=list(range(N_CORES)))
    out = np.stack([np.asarray(res.results[i]["out"]) for i in range(N_CORES)], axis=0)
    return out.astype(np.float32)

